# revision 12
# baseline (speedup 1.0000x reference)
"""Trainium2 Bass kernel for nn_DigitConvolutionalModel (v2 configuration).

Model: x[B,784] -> conv3x3(valid, 28x28->26x26) -> flatten -> Linear(676,256)
       -> relu -> Linear(256,10).

Conv folded into the first Linear on the host; pure data parallelism over
the batch dim across 8 NeuronCores; bf16 compute with fp32 PSUM
accumulation; layer 2 packed 4 groups wide into PE column groups.
"""

import sys

if "/opt/trn_rl_repo" not in sys.path:
    sys.path.insert(0, "/opt/trn_rl_repo")

import ml_dtypes
import numpy as np

B = 65536
NCORES = 8
BC = B // NCORES
P = 128
KC = 7
NF1 = 256
NO = 10
NB = 512
NGRP = BC // NB
NWARM = 26

_PROG = None


def _build_program():
    import concourse.tile as tile
    from concourse import bacc, mybir

    bf16 = mybir.dt.bfloat16
    f32 = mybir.dt.float32

    nc = bacc.Bacc("TRN2", target_bir_lowering=False, debug=False,
                   num_devices=NCORES)
    xt = nc.dram_tensor("xt", [P, NGRP, KC, NB], bf16,
                        kind="ExternalInput").ap()
    w1 = nc.dram_tensor("w1", [P, KC, NF1], bf16, kind="ExternalInput").ap()
    w2 = nc.dram_tensor("w2", [P, 2, NO], bf16, kind="ExternalInput").ap()
    b1 = nc.dram_tensor("b1", [P, 2], f32, kind="ExternalInput").ap()
    b2 = nc.dram_tensor("b2", [P, 1], f32, kind="ExternalInput").ap()
    out = nc.dram_tensor("out", [NO, BC], f32, kind="ExternalOutput").ap()

    with tile.TileContext(nc) as tc:
        with (
            tc.tile_pool(name="singles", bufs=1) as singles,
            tc.tile_pool(name="hp", bufs=12) as hp,
            tc.tile_pool(name="op", bufs=3) as op,
            tc.tile_pool(name="ps1", bufs=4, space="PSUM") as ps1p,
            tc.tile_pool(name="ps2", bufs=2, space="PSUM") as ps2p,
        ):
            wsb = singles.tile([P, P], bf16)
            nc.vector.memset(wsb, 0.0)
            wp = ps2p.tile([32, P], f32, tag="ps2", name="warm")
            for i in range(NWARM):
                nc.tensor.matmul(wp, wsb[:, :32], wsb,
                                 start=(i == 0), stop=(i == NWARM - 1))

            w1sb = singles.tile([P, KC, NF1], bf16)
            nc.scalar.dma_start(out=w1sb[:, :1], in_=w1[:, :1])
            nc.scalar.dma_start(out=w1sb[:, 1:], in_=w1[:, 1:])
            b1sb = singles.tile([P, 2], f32)
            nc.scalar.dma_start(out=b1sb, in_=b1)
            w2sb = singles.tile([P, 2, NO], bf16)
            nc.scalar.dma_start(out=w2sb, in_=w2)
            b2sb = singles.tile([P, 1], f32)
            nc.scalar.dma_start(out=b2sb, in_=b2)

            xsb = singles.tile([P, NGRP, KC, NB], bf16)
            for q in range(4):
                cs = slice(q * 128, (q + 1) * 128)
                nc.sync.dma_start(out=xsb[:, 0, :, cs], in_=xt[:, 0, :, cs])
            for hh in range(2):
                cs = slice(hh * 256, (hh + 1) * 256)
                nc.sync.dma_start(out=xsb[:, 1, :, cs], in_=xt[:, 1, :, cs])
            for g in range(2, NGRP):
                nc.sync.dma_start(out=xsb[:, g], in_=xt[:, g])

            hs_all = []

            def layer1(g, nsplit):
                pss = [ps1p.tile([P, NB], f32, tag="ps1",
                                 name=f"ps1_{g}_{m}") for m in range(2)]
                nw = NB // nsplit
                for s in range(nsplit):
                    cs = slice(s * nw, (s + 1) * nw)
                    for k in range(KC):
                        for m in range(2):
                            nc.tensor.matmul(
                                pss[m][:, cs],
                                w1sb[:, k, m * P:(m + 1) * P],
                                xsb[:, g, k, cs],
                                start=(k == 0),
                                stop=(k == KC - 1),
                            )
                hs = []
                for m in range(2):
                    h = hp.tile([P, NB], bf16, tag="h", name=f"h_{g}_{m}")
                    nc.scalar.activation(
                        h, pss[m], mybir.ActivationFunctionType.Relu,
                        bias=b1sb[:, m:m + 1],
                    )
                    hs.append(h)
                hs_all.append(hs)

            def layer2_pack(gs):
                ps2 = ps2p.tile([P, NB], f32, tag="ps2",
                                name=f"ps2_{gs[0]}")
                for k2 in range(2):
                    for j, g in enumerate(gs):
                        nc.tensor.matmul(
                            ps2[32 * j:32 * j + NO, :],
                            w2sb[:, k2, :],
                            hs_all[g][k2],
                            start=(k2 == 0), stop=(k2 == 1),
                            tile_position=(0, 32 * j),
                        )
                osb = op.tile([P, NB], f32, tag="o", name=f"o_{gs[0]}")
                nc.vector.tensor_scalar_add(osb, ps2, b2sb)
                for j, g in enumerate(gs):
                    nc.sync.dma_start(
                        out=out[:, g * NB:(g + 1) * NB],
                        in_=osb[32 * j:32 * j + NO, :])

            layer1(0, 4)
            layer1(1, 2)
            for g in range(2, NGRP - 1):
                layer1(g, 1)
                if g == 4:
                    layer2_pack([0, 1, 2, 3])
                elif g == 8:
                    layer2_pack([4, 5, 6, 7])
                elif g == 12:
                    layer2_pack([8, 9, 10, 11])

            gl = NGRP - 1
            NH = NB // 2
            ps2t = ps2p.tile([P, NB], f32, tag="ps2", name="ps2_tail")
            hls = []
            for sub in range(2):
                cs = slice(sub * NH, (sub + 1) * NH)
                pss = [ps1p.tile([P, NH], f32, tag="ps1",
                                 name=f"ps1_{gl}_{sub}_{m}") for m in range(2)]
                for k in range(KC):
                    for m in range(2):
                        nc.tensor.matmul(
                            pss[m],
                            w1sb[:, k, m * P:(m + 1) * P],
                            xsb[:, gl, k, cs],
                            start=(k == 0),
                            stop=(k == KC - 1),
                        )
                if sub == 0:
                    layer2_pack([12, 13, 14])
                h0 = hp.tile([P, NH], bf16, tag="h", name=f"h_{gl}_{sub}_0")
                nc.scalar.activation(h0, pss[0],
                                     mybir.ActivationFunctionType.Relu,
                                     bias=b1sb[:, 0:1])
                h1 = hp.tile([P, NH], bf16, tag="h", name=f"h_{gl}_{sub}_1")
                nc.vector.tensor_scalar(h1, pss[1], b1sb[:, 1:2], 0.0,
                                        mybir.AluOpType.add,
                                        mybir.AluOpType.max)
                hls.append((h0, h1))
            for sub in range(2):
                for k2 in range(2):
                    nc.tensor.matmul(
                        ps2t[32 * sub:32 * sub + NO, :NH],
                        w2sb[:, k2, :],
                        hls[sub][k2],
                        start=(k2 == 0), stop=(k2 == 1),
                        tile_position=(0, 32 * sub),
                    )
            osbt = op.tile([P, NH], f32, tag="o", name="o_tail")
            nc.vector.tensor_scalar_add(osbt[:42], ps2t[:42, :NH], b2sb[:42])
            nc.scalar.dma_start(
                out=out[:, gl * NB:gl * NB + NH], in_=osbt[0:NO, :])
            nc.sync.dma_start(
                out=out[:, gl * NB + NH:(gl + 1) * NB], in_=osbt[32:32 + NO, :])

    nc.compile()
    return nc


def _fold_weights(conv_w, W1):
    cw = conv_w.astype(np.float64)
    W1r = W1.astype(np.float64).reshape(NF1, 26, 26).transpose(1, 2, 0)
    W1eff = np.zeros((28, 28, NF1), np.float64)
    for dr in range(3):
        for dc in range(3):
            W1eff[dr:dr + 26, dc:dc + 26, :] += cw[dr, dc] * W1r
    return W1eff.reshape(784, NF1)


def _prep_inputs(x, conv_w, W1, b1, W2, b2):
    bf16 = ml_dtypes.bfloat16
    W1eff = _fold_weights(conv_w, W1)
    w1p = np.zeros((KC * P, NF1), np.float64)
    w1p[:784] = W1eff
    w1p = np.ascontiguousarray(
        w1p.reshape(KC, P, NF1).transpose(1, 0, 2)).astype(bf16)
    w2p = np.ascontiguousarray(
        W2.T.astype(np.float32).reshape(2, P, NO).transpose(1, 0, 2)).astype(bf16)
    b1p = np.ascontiguousarray(b1.astype(np.float32).reshape(2, P).T)
    b2p = np.zeros((P, 1), np.float32)
    for j in range(4):
        b2p[32 * j:32 * j + NO, 0] = b2.astype(np.float32)

    in_maps = []
    for c in range(NCORES):
        xc = np.zeros((KC * P, BC), bf16)
        xcT = np.ascontiguousarray(x[c * BC:(c + 1) * BC].T)
        xc[:784] = xcT.astype(bf16)
        xdev = xc.reshape(KC, P, NGRP, NB).transpose(1, 2, 0, 3)
        in_maps.append({
            "xt": np.ascontiguousarray(xdev),
            "w1": w1p, "w2": w2p, "b1": b1p, "b2": b2p,
        })
    return in_maps


def kernel(x, conv_w, W1, b1, W2, b2, _trace=False, _trace_kwargs=None):
    global _PROG
    from concourse import bass_utils

    x = np.asarray(x, dtype=np.float32)
    conv_w = np.asarray(conv_w, dtype=np.float32)
    W1 = np.asarray(W1, dtype=np.float32)
    b1 = np.asarray(b1, dtype=np.float32)
    W2 = np.asarray(W2, dtype=np.float32)
    b2 = np.asarray(b2, dtype=np.float32)
    assert x.shape == (B, 784), x.shape

    if _PROG is None:
        _PROG = _build_program()

    in_maps = _prep_inputs(x, conv_w, W1, b1, W2, b2)
    kwargs = dict(_trace_kwargs or {})
    res = bass_utils.run_bass_kernel_spmd(
        _PROG, in_maps, core_ids=list(range(NCORES)), trace=_trace, **kwargs)

    out = np.empty((B, NO), np.float32)
    for c in range(NCORES):
        out[c * BC:(c + 1) * BC] = res.results[c]["out"].T
    if _trace:
        return out, res
    return out
